# revision 3
# baseline (speedup 1.0000x reference)
"""Distributed selected-row matvec kernel for nn_CubicalModel_ISM.

The reference computes Xp = I @ p, Yp = J @ p (I, J: [784, 50000]) and then
gathers 100 pixels from each 28x28 image to form two [50, 2] diagrams. Only
the <=100 gathered rows of I and <=100 rows of J can influence the output, so
the device kernel computes exactly those 200 dot products (dead-code
elimination of the other ~584 rows), cutting HBM traffic ~4x vs the full
matvec; bf16 (vs fp32) halves it again (norm-rel error ~1.5e-3, tolerance
2e-2). p stays split as bf16 hi + bf16 lo, which rides in the same matmul.

Strategy (8 NeuronCores):
  - Host: rows = inds[:,0]*28 + inds[:,1]; Xsel = vstack(I[rows1], J[rows2])
    -> [200, 50000] bf16. Shard the contraction dim P across 8 cores (6250
    each, zero-padded to 6400 = 50 k-subtiles of 128); per-core plane is the
    shard transposed to [6400, 200] and packed so each of 5 DMAs moves a
    contiguous [128 x 4000B] block (512 KB).
  - The 5 x-tile DMAs are CHAINED (tile n embeds a wait on tile n-1's
    semaphore): tiles complete strictly in order, each at full 16-engine
    DMA bandwidth, so matmuls on tile 0 overlap the tail tiles' transfers.
    Issue is split across two HWDGE queues (sync: x0,x2,x4; scalar: pw,x1,x3)
    to halve descriptor-generation latency.
  - PE: 50 accumulating matmuls [128,2]^T @ [128,200] into one fp32 PSUM
    bank. The PE p-state ramps to full clock only after ~3us of CONTINUOUS
    execution, so scratch warm-up matmuls run from the entry barrier until
    the first tile lands and between groups, keeping the PE hot (83ns/matmul
    instead of 185ns).
  - Output: one [2,200] f32 DMA pre-armed on the sync queue with an embedded
    wait on pe_sem, reading PSUM directly (no DVE eviction) if the build
    allows, else via a DVE copy.
  - Host: sum the 8 cores' (hi, lo) partial rows in float64, reshape to the
    two [50, 2] diagrams.
"""

import numpy as np
import ml_dtypes

import concourse.bass as bass
import concourse.mybir as mybir
from concourse.bass_utils import run_bass_kernel_spmd

N_CORES = 8
P_FULL = 50000
H = W = 28
NSEL = 200  # selected rows: 100 from I + 100 from J
K_PER = P_FULL // N_CORES  # 6250
KT = 50  # k-subtiles of 128 per core (6400 padded)
K_PAD = KT * 128  # 6400
G = 10  # k-subtiles packed per DMA tile
NT = KT // G  # 5 input tiles
TC = G * NSEL  # 2000 bf16 cols per tile

W0 = 24  # PE warm-up matmuls before the first tile
WG = 6  # PE warm-up matmuls between real groups

BF16 = ml_dtypes.bfloat16
F32 = np.float32


def build_nc(psum_direct_out: bool = True) -> bass.Bass:
    f32 = mybir.dt.float32
    bf16 = mybir.dt.bfloat16
    nc = bass.Bass("TRN2")
    pw_d = nc.dram_tensor("pw", [128, 2 * KT], bf16, kind="ExternalInput")
    x_d = nc.dram_tensor("x", [NT * 128, TC], bf16, kind="ExternalInput")
    out_d = nc.dram_tensor("out", [2, NSEL], f32, kind="ExternalOutput")

    x_tiled = x_d[:, :].rearrange("(n p) m -> n p m", p=128)

    from contextlib import ExitStack

    with ExitStack() as stk:
        pw_sb = stk.enter_context(nc.sbuf_tensor("pw_sb", [128, 2 * KT], bf16))
        x_sb = stk.enter_context(nc.sbuf_tensor("x_sb", [128, NT * TC], bf16))
        scr = stk.enter_context(nc.sbuf_tensor("scr", [128, NSEL], bf16))
        o_sb = stk.enter_context(nc.sbuf_tensor("o_sb", [2, NSEL], f32))
        ps = stk.enter_context(nc.psum_tensor("ps", [2, NSEL], f32))
        wm = stk.enter_context(nc.psum_tensor("wm", [2, NSEL], f32))
        d_pw = stk.enter_context(nc.semaphore("d_pw"))
        d_x = [stk.enter_context(nc.semaphore(f"d_x{n}")) for n in range(NT)]
        d_out = stk.enter_context(nc.semaphore("d_out"))
        pe_sem = stk.enter_context(nc.semaphore("pe_sem"))
        dve_sem = stk.enter_context(nc.semaphore("dve_sem"))
        block = stk.enter_context(nc.Block(no_gpsimd_drain=True))

        def chain_dma(eng, dst, src, done_sem, prev_sem):
            ins = eng.dma_start(dst, src).then_inc(done_sem, 16)
            if prev_sem is not None:
                ins.wait_op(prev_sem, 16, "sem-ge")
            return ins

        def x_dma(eng, n):
            chain_dma(
                eng,
                x_sb[:, n * TC : (n + 1) * TC],
                x_tiled[n, :, :],
                d_x[n],
                d_x[n - 1] if n > 0 else None,
            )

        @block.sync
        def _(sync):
            for n in (0, 2, 4):
                x_dma(sync, n)
            # pre-armed output DMA: descriptors generated now, fires on pe_sem
            out_src = ps if psum_direct_out else o_sb
            ins = sync.dma_start(out_d[:, :], out_src[:, :]).then_inc(d_out, 16)
            ins.wait_op(pe_sem if psum_direct_out else dve_sem, 1, "sem-ge")
            sync.wait_ge(d_out, 16)

        @block.scalar
        def _(scalar):
            scalar.dma_start(pw_sb[:, :], pw_d[:, :]).then_inc(d_pw, 16)
            for n in (1, 3):
                x_dma(scalar, n)

        @block.tensor
        def _(tensor):
            def warmup(k):
                for _ in range(k):
                    nc.tensor.matmul(
                        wm[:, :], pw_sb[:, 0:2], scr[:, :], start=True, stop=True
                    )

            warmup(W0)
            tensor.wait_ge(d_pw, 16)
            for n in range(NT):
                tensor.wait_ge(d_x[n], 16)
                for g in range(G):
                    s = n * G + g
                    ins = nc.tensor.matmul(
                        ps[:, :],
                        pw_sb[:, 2 * s : 2 * s + 2],
                        x_sb[:, n * TC + g * NSEL : n * TC + (g + 1) * NSEL],
                        start=(s == 0),
                        stop=(s == KT - 1),
                    )
                if n < NT - 1:
                    warmup(WG)
            ins.then_inc(pe_sem, 1)

        if not psum_direct_out:

            @block.vector
            def _(vector):
                vector.wait_ge(pe_sem, 1)
                nc.vector.tensor_copy(o_sb[:, :], ps[:, :]).then_inc(dve_sem, 1)

    return nc


_NC_CACHE = None


def get_nc() -> bass.Bass:
    global _NC_CACHE
    if _NC_CACHE is None:
        try:
            _NC_CACHE = build_nc(psum_direct_out=True)
        except Exception:
            _NC_CACHE = build_nc(psum_direct_out=False)
    return _NC_CACHE


def shard_inputs(p, I, J, inds1, inds2) -> list[dict]:
    p = np.asarray(p, dtype=F32)
    I = np.asarray(I, dtype=F32)
    J = np.asarray(J, dtype=F32)
    inds1 = np.asarray(inds1).astype(np.int64)
    inds2 = np.asarray(inds2).astype(np.int64)

    rows1 = inds1[:, 0] * W + inds1[:, 1]
    rows2 = inds2[:, 0] * W + inds2[:, 1]
    xsel = np.concatenate([I[rows1], J[rows2]], axis=0)  # [200, 50000]
    xsel_b = xsel.astype(BF16)

    in_maps = []
    for c in range(N_CORES):
        lo_k = c * K_PER
        pc = np.zeros(K_PAD, dtype=F32)
        pc[:K_PER] = p[lo_k : lo_k + K_PER]
        phi = pc.astype(BF16)
        plo = (pc - phi.astype(F32)).astype(BF16)
        pw = np.zeros((128, 2 * KT), dtype=BF16)
        pw[:, 0::2] = phi.reshape(KT, 128).T
        pw[:, 1::2] = plo.reshape(KT, 128).T

        plane = np.zeros((K_PAD, NSEL), dtype=BF16)
        plane[:K_PER] = xsel_b[:, lo_k : lo_k + K_PER].T
        packed = np.ascontiguousarray(
            plane.reshape(NT, G, 128, NSEL)
            .transpose(0, 2, 1, 3)
            .reshape(NT * 128, TC)
        )
        in_maps.append({"pw": pw, "x": packed})
    return in_maps


def run(p, I, J, inds1, inds2, trace=False, **run_kwargs):
    """Returns ((dgm1, dgm2), BassKernelResults)."""
    in_maps = shard_inputs(p, I, J, inds1, inds2)
    nc = get_nc()
    res = run_bass_kernel_spmd(
        nc, in_maps, list(range(N_CORES)), trace=trace, **run_kwargs
    )
    acc = np.zeros(NSEL, dtype=np.float64)
    for r in res.results:
        o = r["out"].astype(np.float64)
        acc += o[0] + o[1]
    y = acc.astype(F32)
    dgm1 = y[:100].reshape(-1, 2)
    dgm2 = y[100:].reshape(-1, 2)
    return (dgm1, dgm2), res


def kernel(p, I, J, inds1, inds2):
    out, _ = run(p, I, J, inds1, inds2, trace=False)
    return out


# revision 7
# speedup vs baseline: 1.4578x; 1.4578x over previous
"""Distributed selected-row matvec kernel for nn_CubicalModel_ISM.

The reference computes Xp = I @ p, Yp = J @ p (I, J: [784, 50000]) and then
gathers 100 pixels from each 28x28 image into two [50, 2] diagrams. Only the
<=100 gathered rows of I and <=100 rows of J can influence the output, so the
device kernel computes exactly those 200 dot products (dead-code elimination
of the other ~584 rows), cutting HBM traffic ~4x vs the full matvec; bf16
(vs fp32) halves it again (norm-rel error ~1.5e-3 vs the 2e-2 tolerance).
p is split bf16 hi + bf16 lo and both rows ride in the same matmul, so only
the matrix rounding contributes error.

Per-core device schedule (8 NeuronCores, P=50000 sharded 6250/core, padded
to 6400 = 50 k-subtiles of 128):
  - x plane [6400, 200] bf16 packed as 10 tiles of 5 k-subtiles, each a
    contiguous [128 x 2000B] block (256 KB). Issued concurrently on three
    DMA queues (SP HWDGE: tiles 0,2,4,6,8,9; ACT HWDGE: tiles 1,3,5,7;
    gpsimd SWDGE: just the tiny pw, so it lands before the first real
    matmul) -> ~360 GB/s aggregate, staggered completions.
  - PE: scratch warm-up matmuls bridge the p-state ramp (the PE reaches
    full clock only after ~3us of continuous execution) and fill arrival
    gaps, so the 50 real accumulating matmuls [128,2]^T @ [128,200] run at
    ~85ns each into one fp32 PSUM bank.
  - DVE evicts PSUM -> SBUF; a pre-queued SP DMA with an embedded wait
    ships the [2,200] f32 partial; gpsimd holds program end until the
    output DMA's semaphore confirms completion.
  - Raw Bass, no Block: all instructions are emitted into the main bb,
    which avoids the Block end-of-program drain/barrier round.
Host: sum the 8 cores' (hi, lo) partial rows in float64 and reshape into
the two [50, 2] diagrams.
"""

import numpy as np
import ml_dtypes

import concourse.bass as bass
import concourse.mybir as mybir
from concourse.bass_utils import run_bass_kernel_spmd

N_CORES = 8
P_FULL = 50000
H = W = 28
NSEL = 200  # selected rows: 100 from I + 100 from J
K_PER = P_FULL // N_CORES  # 6250
KT = 50  # k-subtiles of 128 per core (6400 padded)
K_PAD = KT * 128  # 6400
G = 5  # k-subtiles per DMA tile
NT = KT // G  # 10 tiles
TC = G * NSEL  # 1000 bf16 cols per tile

W0 = 52  # PE warm-up matmuls before the first tile
WG = 24  # PE warm-up matmuls after each odd tile
WCOL = 64  # warm-up matmul column count (fine-grained filler)

BF16 = ml_dtypes.bfloat16
F32 = np.float32


def build_nc() -> bass.Bass:
    f32 = mybir.dt.float32
    bf16 = mybir.dt.bfloat16
    nc = bass.Bass("TRN2")
    pw_d = nc.dram_tensor("pw", [128, 2 * KT], bf16, kind="ExternalInput")
    x_d = nc.dram_tensor("x", [NT * 128, TC], bf16, kind="ExternalInput")
    out_d = nc.dram_tensor("out", [2, NSEL], f32, kind="ExternalOutput")
    x_tiled = x_d[:, :].rearrange("(n p) m -> n p m", p=128)

    from contextlib import ExitStack

    with ExitStack() as stk:
        pw_sb = stk.enter_context(nc.sbuf_tensor("pw_sb", [128, 2 * KT], bf16))
        x_sb = stk.enter_context(nc.sbuf_tensor("x_sb", [128, NT * TC], bf16))
        scr = stk.enter_context(nc.sbuf_tensor("scr", [128, NSEL], bf16))
        o_sb = stk.enter_context(nc.sbuf_tensor("o_sb", [2, NSEL], f32))
        ps = stk.enter_context(nc.psum_tensor("ps", [2, NSEL], f32))
        wm = stk.enter_context(nc.psum_tensor("wm", [2, NSEL], f32))
        d_pw = stk.enter_context(nc.semaphore("d_pw"))
        d_x = [stk.enter_context(nc.semaphore(f"d_x{t}")) for t in range(NT)]
        d_out = stk.enter_context(nc.semaphore("d_out"))
        pe_sem = stk.enter_context(nc.semaphore("pe_sem"))
        dve_sem = stk.enter_context(nc.semaphore("dve_sem"))

        def x_dma(eng, t):
            eng.dma_start(
                x_sb[:, t * TC : (t + 1) * TC], x_tiled[t, :, :]
            ).then_inc(d_x[t], 16)

        def pw_dma(eng):
            eng.dma_start(pw_sb[:, :], pw_d[:, :]).then_inc(d_pw, 16)

        for t in (0, 2, 4, 6, 8, 9):
            x_dma(nc.sync, t)
        for t in (1, 3, 5, 7):
            x_dma(nc.scalar, t)
        pw_dma(nc.gpsimd)

        def warmup(k):
            for _ in range(k):
                nc.tensor.matmul(
                    wm[:, 0:WCOL], pw_sb[:, 0:2], scr[:, 0:WCOL],
                    start=True, stop=True,
                )

        warmup(W0)
        nc.tensor.wait_ge(d_pw, 16)
        ins = None
        first = True
        for t in range(NT):
            nc.tensor.wait_ge(d_x[t], 16)
            for g in range(G):
                s = t * G + g
                ins = nc.tensor.matmul(
                    ps[:, :],
                    pw_sb[:, 2 * s : 2 * s + 2],
                    x_sb[:, t * TC + g * NSEL : t * TC + (g + 1) * NSEL],
                    start=first,
                    stop=(s == KT - 1),
                )
                first = False
            if t % 2 == 1 and t < NT - 1:
                warmup(WG)
        ins.then_inc(pe_sem, 1)

        nc.vector.wait_ge(pe_sem, 1)
        nc.vector.tensor_copy(o_sb[:, :], ps[:, :]).then_inc(dve_sem, 1)

        oins = nc.sync.dma_start(out_d[:, :], o_sb[:, :]).then_inc(d_out, 16)
        oins.wait_op(dve_sem, 1, "sem-ge")
        nc.gpsimd.wait_ge(d_out, 16)

    return nc


_NC_CACHE = None


def get_nc() -> bass.Bass:
    global _NC_CACHE
    if _NC_CACHE is None:
        _NC_CACHE = build_nc()
    return _NC_CACHE


def shard_inputs(p, I, J, inds1, inds2) -> list[dict]:
    p = np.asarray(p, dtype=F32)
    I = np.asarray(I, dtype=F32)
    J = np.asarray(J, dtype=F32)
    inds1 = np.asarray(inds1).astype(np.int64)
    inds2 = np.asarray(inds2).astype(np.int64)

    rows1 = inds1[:, 0] * W + inds1[:, 1]
    rows2 = inds2[:, 0] * W + inds2[:, 1]
    xsel = np.concatenate([I[rows1], J[rows2]], axis=0)  # [200, 50000]
    xsel_b = xsel.astype(BF16)

    in_maps = []
    for c in range(N_CORES):
        lo_k = c * K_PER
        pc = np.zeros(K_PAD, dtype=F32)
        pc[:K_PER] = p[lo_k : lo_k + K_PER]
        phi = pc.astype(BF16)
        plo = (pc - phi.astype(F32)).astype(BF16)
        pw = np.zeros((128, 2 * KT), dtype=BF16)
        pw[:, 0::2] = phi.reshape(KT, 128).T
        pw[:, 1::2] = plo.reshape(KT, 128).T

        plane = np.zeros((K_PAD, NSEL), dtype=BF16)
        plane[:K_PER] = xsel_b[:, lo_k : lo_k + K_PER].T
        packed = np.ascontiguousarray(
            plane.reshape(NT, G, 128, NSEL)
            .transpose(0, 2, 1, 3)
            .reshape(NT * 128, TC)
        )
        in_maps.append({"pw": pw, "x": packed})
    return in_maps


def run(p, I, J, inds1, inds2, trace=False, **run_kwargs):
    """Returns ((dgm1, dgm2), BassKernelResults)."""
    in_maps = shard_inputs(p, I, J, inds1, inds2)
    nc = get_nc()
    res = run_bass_kernel_spmd(
        nc, in_maps, list(range(N_CORES)), trace=trace, **run_kwargs
    )
    acc = np.zeros(NSEL, dtype=np.float64)
    for r in res.results:
        o = r["out"].astype(np.float64)
        acc += o[0] + o[1]
    y = acc.astype(F32)
    dgm1 = y[:100].reshape(-1, 2)
    dgm2 = y[100:].reshape(-1, 2)
    return (dgm1, dgm2), res


def kernel(p, I, J, inds1, inds2):
    out, _ = run(p, I, J, inds1, inds2, trace=False)
    return out


# revision 10
# speedup vs baseline: 1.4835x; 1.0177x over previous
"""Distributed selected-row matvec kernel for nn_CubicalModel_ISM.

The reference computes Xp = I @ p, Yp = J @ p (I, J: [784, 50000]) and then
gathers 100 pixels from each 28x28 image into two [50, 2] diagrams. Only the
<=100 gathered rows of I and <=100 rows of J can influence the output, so the
device kernel computes exactly those 200 dot products (dead-code elimination
of the other ~584 rows), cutting HBM traffic ~4x vs the full matvec; bf16
(vs fp32) halves it again (norm-rel error ~1.5e-3 vs the 2e-2 tolerance).
p is split bf16 hi + bf16 lo and both rows ride in the same matmul, so only
the matrix rounding contributes error.

Per-core device schedule (8 NeuronCores, P=50000 sharded 6250/core, padded
to 6400 = 50 k-subtiles of 128):
  - x plane [6400, 200] bf16 packed as 10 tiles of 5 k-subtiles, each a
    contiguous [128 x 2000B] block (256 KB). Issued concurrently on three
    DMA queues (SP HWDGE: tiles 0,2,4,6,8; ACT HWDGE: tiles 1,3,5,7,9;
    gpsimd SWDGE: just the tiny pw, so it lands before the first real
    matmul) -> ~360 GB/s aggregate, staggered completions.
  - PE: scratch warm-up matmuls bridge the p-state ramp (the PE reaches
    full clock only after ~3us of continuous execution) and fill arrival
    gaps, so the 50 real accumulating matmuls [128,2]^T @ [128,200] run at
    ~85ns each into one fp32 PSUM bank.
  - DVE evicts PSUM -> SBUF; a pre-queued SP DMA with an embedded wait
    ships the [2,200] f32 partial; gpsimd holds program end until the
    output DMA's semaphore confirms completion.
  - Raw Bass, no Block: all instructions are emitted into the main bb,
    which avoids the Block end-of-program drain/barrier round.
Host: sum the 8 cores' (hi, lo) partial rows in float64 and reshape into
the two [50, 2] diagrams.
"""

import numpy as np
import ml_dtypes

import concourse.bass as bass
import concourse.mybir as mybir
from concourse.bass_utils import run_bass_kernel_spmd

N_CORES = 8
P_FULL = 50000
H = W = 28
NSEL = 200  # selected rows: 100 from I + 100 from J
K_PER = P_FULL // N_CORES  # 6250
KT = 50  # k-subtiles of 128 per core (6400 padded)
K_PAD = KT * 128  # 6400
G = 5  # k-subtiles per DMA tile
NT = KT // G  # 10 tiles
TC = G * NSEL  # 1000 bf16 cols per tile

W0 = 52  # PE warm-up matmuls before the first tile
WG = 24  # PE warm-up matmuls after each odd tile
WCOL = 64  # warm-up matmul column count (fine-grained filler)

BF16 = ml_dtypes.bfloat16
F32 = np.float32


def build_nc() -> bass.Bass:
    f32 = mybir.dt.float32
    bf16 = mybir.dt.bfloat16
    nc = bass.Bass("TRN2")
    pw_d = nc.dram_tensor("pw", [128, 2 * KT], bf16, kind="ExternalInput")
    x_d = nc.dram_tensor("x", [NT * 128, TC], bf16, kind="ExternalInput")
    out_d = nc.dram_tensor("out", [2, NSEL], f32, kind="ExternalOutput")
    x_tiled = x_d[:, :].rearrange("(n p) m -> n p m", p=128)

    from contextlib import ExitStack

    with ExitStack() as stk:
        pw_sb = stk.enter_context(nc.sbuf_tensor("pw_sb", [128, 2 * KT], bf16))
        x_sb = stk.enter_context(nc.sbuf_tensor("x_sb", [128, NT * TC], bf16))
        scr = stk.enter_context(nc.sbuf_tensor("scr", [128, NSEL], bf16))
        o_sb = stk.enter_context(nc.sbuf_tensor("o_sb", [2, NSEL], f32))
        ps = stk.enter_context(nc.psum_tensor("ps", [2, NSEL], f32))
        wm = stk.enter_context(nc.psum_tensor("wm", [2, NSEL], f32))
        d_pw = stk.enter_context(nc.semaphore("d_pw"))
        d_x = [stk.enter_context(nc.semaphore(f"d_x{t}")) for t in range(NT)]
        d_out = stk.enter_context(nc.semaphore("d_out"))
        pe_sem = stk.enter_context(nc.semaphore("pe_sem"))
        dve_sem = stk.enter_context(nc.semaphore("dve_sem"))

        def x_dma(eng, t):
            eng.dma_start(
                x_sb[:, t * TC : (t + 1) * TC], x_tiled[t, :, :]
            ).then_inc(d_x[t], 16)

        def pw_dma(eng):
            eng.dma_start(pw_sb[:, :], pw_d[:, :]).then_inc(d_pw, 16)

        for t in (0, 2, 4, 6, 8):
            x_dma(nc.sync, t)
        for t in (1, 3, 5, 7, 9):
            x_dma(nc.scalar, t)
        pw_dma(nc.gpsimd)

        def warmup(k):
            for _ in range(k):
                nc.tensor.matmul(
                    wm[:, 0:WCOL], pw_sb[:, 0:2], scr[:, 0:WCOL],
                    start=True, stop=True,
                )

        warmup(W0)
        nc.tensor.wait_ge(d_pw, 16)
        ins = None
        first = True
        for t in range(NT):
            nc.tensor.wait_ge(d_x[t], 16)
            for g in range(G):
                s = t * G + g
                ins = nc.tensor.matmul(
                    ps[:, :],
                    pw_sb[:, 2 * s : 2 * s + 2],
                    x_sb[:, t * TC + g * NSEL : t * TC + (g + 1) * NSEL],
                    start=first,
                    stop=(s == KT - 1),
                )
                first = False
            if t % 2 == 1 and t < NT - 1:
                warmup(WG)
        ins.then_inc(pe_sem, 1)

        nc.vector.wait_ge(pe_sem, 1)
        nc.vector.tensor_copy(o_sb[:, :], ps[:, :]).then_inc(dve_sem, 1)

        # No explicit wait on d_out: the lowering's epilogue (per-engine
        # drains + ~6us of semaphore clears before any queue reset) flushes
        # the SP queue, so the 1.6KB output DMA always lands first. Dropping
        # the gate saves the 0.9us DMA-semaphore propagation at the end.
        oins = nc.sync.dma_start(out_d[:, :], o_sb[:, :]).then_inc(d_out, 16)
        oins.wait_op(dve_sem, 1, "sem-ge")

    return nc


_NC_CACHE = None


def get_nc() -> bass.Bass:
    global _NC_CACHE
    if _NC_CACHE is None:
        _NC_CACHE = build_nc()
    return _NC_CACHE


def shard_inputs(p, I, J, inds1, inds2) -> list[dict]:
    p = np.asarray(p, dtype=F32)
    I = np.asarray(I, dtype=F32)
    J = np.asarray(J, dtype=F32)
    inds1 = np.asarray(inds1).astype(np.int64)
    inds2 = np.asarray(inds2).astype(np.int64)

    rows1 = inds1[:, 0] * W + inds1[:, 1]
    rows2 = inds2[:, 0] * W + inds2[:, 1]
    xsel = np.concatenate([I[rows1], J[rows2]], axis=0)  # [200, 50000]
    xsel_b = xsel.astype(BF16)

    in_maps = []
    for c in range(N_CORES):
        lo_k = c * K_PER
        pc = np.zeros(K_PAD, dtype=F32)
        pc[:K_PER] = p[lo_k : lo_k + K_PER]
        phi = pc.astype(BF16)
        plo = (pc - phi.astype(F32)).astype(BF16)
        pw = np.zeros((128, 2 * KT), dtype=BF16)
        pw[:, 0::2] = phi.reshape(KT, 128).T
        pw[:, 1::2] = plo.reshape(KT, 128).T

        plane = np.zeros((K_PAD, NSEL), dtype=BF16)
        plane[:K_PER] = xsel_b[:, lo_k : lo_k + K_PER].T
        packed = np.ascontiguousarray(
            plane.reshape(NT, G, 128, NSEL)
            .transpose(0, 2, 1, 3)
            .reshape(NT * 128, TC)
        )
        in_maps.append({"pw": pw, "x": packed})
    return in_maps


def run(p, I, J, inds1, inds2, trace=False, **run_kwargs):
    """Returns ((dgm1, dgm2), BassKernelResults)."""
    in_maps = shard_inputs(p, I, J, inds1, inds2)
    nc = get_nc()
    res = run_bass_kernel_spmd(
        nc, in_maps, list(range(N_CORES)), trace=trace, **run_kwargs
    )
    acc = np.zeros(NSEL, dtype=np.float64)
    for r in res.results:
        o = r["out"].astype(np.float64)
        acc += o[0] + o[1]
    y = acc.astype(F32)
    dgm1 = y[:100].reshape(-1, 2)
    dgm2 = y[100:].reshape(-1, 2)
    return (dgm1, dgm2), res


def kernel(p, I, J, inds1, inds2):
    out, _ = run(p, I, J, inds1, inds2, trace=False)
    return out


# revision 11
# speedup vs baseline: 1.4890x; 1.0037x over previous
"""Distributed selected-row matvec kernel for nn_CubicalModel_ISM.

The reference computes Xp = I @ p, Yp = J @ p (I, J: [784, 50000]) and then
gathers 100 pixels from each 28x28 image into two [50, 2] diagrams. Only the
<=100 gathered rows of I and <=100 rows of J can influence the output, so the
device kernel computes exactly those 200 dot products (dead-code elimination
of the other ~584 rows), cutting HBM traffic ~4x vs the full matvec; bf16
(vs fp32) halves it again (norm-rel error ~1.5e-3 vs the 2e-2 tolerance).
p is split bf16 hi + bf16 lo and both rows ride in the same matmul, so only
the matrix rounding contributes error.

Per-core device schedule (8 NeuronCores, P=50000 sharded 6250/core, padded
to 6400 = 50 k-subtiles of 128):
  - x plane [6400, 200] bf16 packed as 10 tiles of 5 k-subtiles, each a
    contiguous [128 x 2000B] block (256 KB). Issued concurrently on three
    DMA queues (SP HWDGE: tiles 0,2,4,6,8; ACT HWDGE: tiles 1,3,5,7,9;
    gpsimd SWDGE: just the tiny pw, so it lands before the first real
    matmul) -> ~360 GB/s aggregate, staggered completions.
  - PE: scratch warm-up matmuls bridge the p-state ramp (the PE reaches
    full clock only after ~3us of continuous execution) and fill arrival
    gaps, so the 50 real accumulating matmuls [128,2]^T @ [128,200] run at
    ~85ns each into one fp32 PSUM bank.
  - DVE evicts PSUM -> SBUF; a pre-queued SP DMA with an embedded wait
    ships the [2,200] f32 partial; gpsimd holds program end until the
    output DMA's semaphore confirms completion.
  - Raw Bass, no Block: all instructions are emitted into the main bb,
    which avoids the Block end-of-program drain/barrier round.
Host: sum the 8 cores' (hi, lo) partial rows in float64 and reshape into
the two [50, 2] diagrams.
"""

import numpy as np
import ml_dtypes

import concourse.bass as bass
import concourse.mybir as mybir
from concourse.bass_utils import run_bass_kernel_spmd

N_CORES = 8
P_FULL = 50000
H = W = 28
NSEL = 200  # selected rows: 100 from I + 100 from J
K_PER = P_FULL // N_CORES  # 6250
KT = 50  # k-subtiles of 128 per core (6400 padded)
K_PAD = KT * 128  # 6400
G = 5  # k-subtiles per DMA tile
NT = KT // G  # 10 tiles
TC = G * NSEL  # 1000 bf16 cols per tile

W0 = 52  # PE warm-up matmuls before the first tile
WG = 24  # PE warm-up matmuls after each odd tile
WCOL = 64  # warm-up matmul column count (fine-grained filler)

BF16 = ml_dtypes.bfloat16
F32 = np.float32


def build_nc() -> bass.Bass:
    f32 = mybir.dt.float32
    bf16 = mybir.dt.bfloat16
    nc = bass.Bass("TRN2")
    pw_d = nc.dram_tensor("pw", [128, 2 * KT], bf16, kind="ExternalInput")
    x_d = nc.dram_tensor("x", [NT * 128, TC], bf16, kind="ExternalInput")
    out_d = nc.dram_tensor("out", [2, NSEL], f32, kind="ExternalOutput")
    x_tiled = x_d[:, :].rearrange("(n p) m -> n p m", p=128)

    from contextlib import ExitStack

    with ExitStack() as stk:
        pw_sb = stk.enter_context(nc.sbuf_tensor("pw_sb", [128, 2 * KT], bf16))
        x_sb = stk.enter_context(nc.sbuf_tensor("x_sb", [128, NT * TC], bf16))
        scr = stk.enter_context(nc.sbuf_tensor("scr", [128, NSEL], bf16))
        o_sb = stk.enter_context(nc.sbuf_tensor("o_sb", [2, NSEL], f32))
        ps = stk.enter_context(nc.psum_tensor("ps", [2, NSEL], f32))
        wm = stk.enter_context(nc.psum_tensor("wm", [2, NSEL], f32))
        d_pw = stk.enter_context(nc.semaphore("d_pw"))
        d_x = [stk.enter_context(nc.semaphore(f"d_x{t}")) for t in range(NT)]
        d_out = stk.enter_context(nc.semaphore("d_out"))
        pe_sem = stk.enter_context(nc.semaphore("pe_sem"))
        dve_sem = stk.enter_context(nc.semaphore("dve_sem"))

        def x_dma(eng, t):
            eng.dma_start(
                x_sb[:, t * TC : (t + 1) * TC], x_tiled[t, :, :]
            ).then_inc(d_x[t], 16)

        def pw_dma(eng):
            eng.dma_start(pw_sb[:, :], pw_d[:, :]).then_inc(d_pw, 16)

        for t in (0, 2, 4, 6, 8):
            x_dma(nc.sync, t)
        for t in (1, 3, 5, 7, 9):
            x_dma(nc.scalar, t)
        pw_dma(nc.gpsimd)

        def warmup(k):
            for _ in range(k):
                nc.tensor.matmul(
                    wm[:, 0:WCOL], pw_sb[:, 0:2], scr[:, 0:WCOL],
                    start=True, stop=True,
                )

        warmup(W0)
        nc.tensor.wait_ge(d_pw, 16)
        ins = None
        first = True
        for t in range(NT):
            nc.tensor.wait_ge(d_x[t], 16)
            for g in range(G):
                s = t * G + g
                ins = nc.tensor.matmul(
                    ps[:, :],
                    pw_sb[:, 2 * s : 2 * s + 2],
                    x_sb[:, t * TC + g * NSEL : t * TC + (g + 1) * NSEL],
                    start=first,
                    stop=(s == KT - 1),
                )
                first = False
            if t % 2 == 1 and t < NT - 1:
                warmup(WG)
        ins.then_inc(pe_sem, 1)

        nc.vector.wait_ge(pe_sem, 1)
        nc.vector.tensor_copy(o_sb[:, :], ps[:, :]).then_inc(dve_sem, 1)

        # No explicit wait on d_out: the lowering's epilogue (per-engine
        # drains + ~6us of semaphore clears before any queue reset) flushes
        # the SP queue, so the 1.6KB output DMA always lands first. Dropping
        # the gate saves the 0.9us DMA-semaphore propagation at the end, and
        # an explicit engine wait is ~0.8us cheaper than embedding the wait
        # in the DMA instruction.
        nc.sync.wait_ge(dve_sem, 1)
        nc.sync.dma_start(out_d[:, :], o_sb[:, :]).then_inc(d_out, 16)

    return nc


_NC_CACHE = None


def get_nc() -> bass.Bass:
    global _NC_CACHE
    if _NC_CACHE is None:
        _NC_CACHE = build_nc()
    return _NC_CACHE


def shard_inputs(p, I, J, inds1, inds2) -> list[dict]:
    p = np.asarray(p, dtype=F32)
    I = np.asarray(I, dtype=F32)
    J = np.asarray(J, dtype=F32)
    inds1 = np.asarray(inds1).astype(np.int64)
    inds2 = np.asarray(inds2).astype(np.int64)

    rows1 = inds1[:, 0] * W + inds1[:, 1]
    rows2 = inds2[:, 0] * W + inds2[:, 1]
    xsel = np.concatenate([I[rows1], J[rows2]], axis=0)  # [200, 50000]
    xsel_b = xsel.astype(BF16)

    in_maps = []
    for c in range(N_CORES):
        lo_k = c * K_PER
        pc = np.zeros(K_PAD, dtype=F32)
        pc[:K_PER] = p[lo_k : lo_k + K_PER]
        phi = pc.astype(BF16)
        plo = (pc - phi.astype(F32)).astype(BF16)
        pw = np.zeros((128, 2 * KT), dtype=BF16)
        pw[:, 0::2] = phi.reshape(KT, 128).T
        pw[:, 1::2] = plo.reshape(KT, 128).T

        plane = np.zeros((K_PAD, NSEL), dtype=BF16)
        plane[:K_PER] = xsel_b[:, lo_k : lo_k + K_PER].T
        packed = np.ascontiguousarray(
            plane.reshape(NT, G, 128, NSEL)
            .transpose(0, 2, 1, 3)
            .reshape(NT * 128, TC)
        )
        in_maps.append({"pw": pw, "x": packed})
    return in_maps


def run(p, I, J, inds1, inds2, trace=False, **run_kwargs):
    """Returns ((dgm1, dgm2), BassKernelResults)."""
    in_maps = shard_inputs(p, I, J, inds1, inds2)
    nc = get_nc()
    res = run_bass_kernel_spmd(
        nc, in_maps, list(range(N_CORES)), trace=trace, **run_kwargs
    )
    acc = np.zeros(NSEL, dtype=np.float64)
    for r in res.results:
        o = r["out"].astype(np.float64)
        acc += o[0] + o[1]
    y = acc.astype(F32)
    dgm1 = y[:100].reshape(-1, 2)
    dgm2 = y[100:].reshape(-1, 2)
    return (dgm1, dgm2), res


def kernel(p, I, J, inds1, inds2):
    out, _ = run(p, I, J, inds1, inds2, trace=False)
    return out
